# revision 21
# baseline (speedup 1.0000x reference)
"""ColBERT negative-CE loss on 8 Trainium2 NeuronCores (Bass/Tile).

Problem (hardcoded shapes): B=64, N=32 query tokens, S=1024 doc tokens, D=128.
  pos/neg paired MaxSim + in-batch (b x c) MaxSim cross-entropy, T=0.02.

Strategy (v2):
  * Shard the in-batch score matrix by DOC COLUMNS: core r computes
    scores[:, r*8:(r+1)*8] (all 64 query rows vs its 8 docs) plus the paired
    neg scores for its own 8 batch rows (~9 MB of input per core).
  * Per-doc max over 1024 tokens via max(a,b) = (a+b)/2 + |a-b|/2 on the
    512-token halves: host precomputes hsum=(dA+dB)/2, hdif=(dA-dB)/2;
    PE computes P = q@hsum, Q = q@hdif; ScalarE takes |Q|; PE accumulates it
    onto P with an identity matmul; VectorE max-reduces the merged 512.
  * v2 layout: one 4-bank PSUM allocation [128, 2048] holds TWO tiles as
    P_a|P_b|Q_a|Q_b, so ScalarE runs ONE mega-abs over [128,1024] and
    VectorE ONE mega-reduce over [128,2,512] per allocation — halving the
    per-instruction overheads that dominated v1 (ABS 659ns, MAX 680ns per
    512 columns vs ~427/533 streaming cost).
  * The neg term packs 4 batch rows (4x32 query tokens) into the full 128
    partitions of each bank, so its abs/merge/reduce run at full width
    (v1 wasted 96 of 128 partitions on [32,512] tiles).
  * No on-device token-sum: the per-(token, doc) maxes [128, 130] are DMA'd
    out and the O(B^2) epilogue (token sums, softmax CE, softplus) runs on
    host in float64.
"""

import numpy as np

B = 64
N = 32  # query tokens per row
S = 1024  # doc tokens
D = 128
NCORES = 8
LB = B // NCORES  # 8 docs (and batch rows) per core
H = S // 2  # 512, half-doc
MT = (B * N) // 128  # 16 m-tiles of 128 query tokens
TEMP = 0.02
NWAVE = LB // 2  # 4 doc-pair waves
OUT_COLS = MT * LB + 8  # 128 in-batch cols + 8 neg cols = 136

_NC_CACHE = {}


def _build_nc():
    import concourse.bacc as bacc
    import concourse.mybir as mybir
    import concourse.tile as tile

    F32 = mybir.dt.float32
    F32R = mybir.dt.float32r
    X = mybir.AxisListType.X
    ABS = mybir.ActivationFunctionType.Abs

    nc = bacc.Bacc("TRN2", target_bir_lowering=False, debug=False)

    qT = nc.dram_tensor("qT", [128, B * N], F32, kind="ExternalInput").ap()
    qLocT = nc.dram_tensor("qLocT", [128, LB * N], F32, kind="ExternalInput").ap()
    dsumT = nc.dram_tensor("dsumT", [128, LB * H], F32, kind="ExternalInput").ap()
    ddifT = nc.dram_tensor("ddifT", [128, LB * H], F32, kind="ExternalInput").ap()
    nsumT = nc.dram_tensor("nsumT", [128, LB * H], F32, kind="ExternalInput").ap()
    ndifT = nc.dram_tensor("ndifT", [128, LB * H], F32, kind="ExternalInput").ap()
    iden = nc.dram_tensor("iden", [128, 128], F32, kind="ExternalInput").ap()
    out = nc.dram_tensor("out", [128, OUT_COLS], F32, kind="ExternalOutput").ap()

    with tile.TileContext(nc) as tc:
        with (
            tc.tile_pool(name="consts", bufs=1) as consts,
            tc.tile_pool(name="docs", bufs=1) as docs_p,
            tc.tile_pool(name="negs", bufs=1) as negs_p,
            tc.tile_pool(name="absq", bufs=3) as absq_p,
            tc.tile_pool(name="ttro", bufs=2) as ttro_p,
            tc.tile_pool(name="psump", bufs=2, space="PSUM") as psum_p,
            tc.tile_pool(name="psumq", bufs=2, space="PSUM") as psum_q,
        ):
            # queries split into 4 chunks so compute can start after chunk 0
            q_t = []
            for g in range(4):
                t = consts.tile([128, 512], F32R, tag=f"q{g}")
                q_t.append(t)
            id_t = consts.tile([128, 128], F32R, tag="id")
            ql_t = consts.tile([128, LB * N], F32R, tag="ql")
            mx = consts.tile([128, OUT_COLS], F32, tag="mx")

            # HAM warm-up: dummy matmuls on memset data while input DMAs are
            # still in flight, so real matmuls start at the full PE clock
            wa = consts.tile([128, 128], F32, tag="wa")
            nc.vector.memset(wa[:], 0.0)
            wps = psum_q.tile([128, 1024], F32, tag="qq", name="warm")
            for _ in range(12):
                nc.tensor.matmul(wps[:, 0:128], wa[:], wa[:], start=True, stop=True)

            # doc tiles: first pair separate (fast first dependency), rest as
            # big chunks; negs as one chunk per tensor (needed last).
            ds0 = docs_p.tile([128, 2 * H], F32R, tag="ds0")
            dd0 = docs_p.tile([128, 2 * H], F32R, tag="dd0")
            dsA = docs_p.tile([128, 6 * H], F32R, tag="dsA")
            ddA = docs_p.tile([128, 6 * H], F32R, tag="ddA")
            nsr = negs_p.tile([128, LB * H], F32R, tag="nsr")
            ndr = negs_p.tile([128, LB * H], F32R, tag="ndr")

            def ds_ap(c):
                if c < 2:
                    return ds0[:, c * H : (c + 1) * H]
                return dsA[:, (c - 2) * H : (c - 1) * H]

            def dd_ap(c):
                if c < 2:
                    return dd0[:, c * H : (c + 1) * H]
                return ddA[:, (c - 2) * H : (c - 1) * H]

            nc.sync.dma_start(ds0[:], dsumT[:, 0 : 2 * H].bitcast(F32R))
            nc.sync.dma_start(dd0[:], ddifT[:, 0 : 2 * H].bitcast(F32R))
            nc.sync.dma_start(q_t[0][:], qT[:, 0:512].bitcast(F32R))
            nc.sync.dma_start(id_t[:], iden[:].bitcast(F32R))
            for g in range(1, 4):
                nc.sync.dma_start(q_t[g][:], qT[:, g * 512 : (g + 1) * 512].bitcast(F32R))
            nc.sync.dma_start(dsA[:], dsumT[:, 2 * H : 8 * H].bitcast(F32R))
            nc.sync.dma_start(ddA[:], ddifT[:, 2 * H : 8 * H].bitcast(F32R))
            nc.sync.dma_start(ql_t[:], qLocT[:].bitcast(F32R))
            nc.sync.dma_start(nsr[:], nsumT[:].bitcast(F32R))
            nc.sync.dma_start(ndr[:], ndifT[:].bitcast(F32R))

            # in-batch term: per iteration (doc pair a=2w, b=2w+1; m-tile m):
            # Q-pool alloc (freed after the mega-abs/copy, 2-stage lifetime)
            # and P-pool alloc (freed after merge+reduce / ttr).
            # Waves 0-1 (docs 0-3, host-prepped sum/dif halves): ScalarE abs,
            #   PE identity-merge, VectorE mega-reduce — the max(a,b) =
            #   (a+b)/2 + |a-b|/2 trick.
            # Waves 2-3 (docs 4-7, raw halves): ScalarE copies R to SBUF and
            #   VectorE computes max(L,R) + max-reduce in ONE fused
            #   tensor_tensor_reduce — no PE merge matmuls. This trades the
            #   PE's merge work (the bottleneck) for VectorE slack.
            # Q fills first so abs/copy starts as early as possible; merge/
            # reduce deferred one iteration so the PE never idles on ScalarE.
            MAX = mybir.AluOpType.max
            pend = []

            def flush_pend():
                kind, pt, aq, col = pend.pop(0)
                if kind == 1:
                    # merge |Q| onto P for both tiles, then mega-reduce
                    nc.tensor.matmul(
                        pt[:, 0:512], id_t[:], aq[:, 0:512], start=False, stop=True
                    )
                    nc.tensor.matmul(
                        pt[:, 512:1024], id_t[:], aq[:, 512:1024], start=False, stop=True
                    )
                    nc.vector.reduce_max(
                        mx[:, col : col + 2],
                        pt[:].rearrange("p (g k) -> p g k", g=2),
                        axis=X,
                    )
                else:
                    for j in range(2):
                        to = ttro_p.tile([128, 512], F32, tag="to")
                        nc.vector.tensor_tensor_reduce(
                            to[:],
                            pt[:, j * 512 : (j + 1) * 512],
                            aq[:, j * 512 : (j + 1) * 512].bitcast(F32),
                            1.0,
                            -3.0e38,
                            MAX,
                            MAX,
                            mx[:, col + j : col + j + 1],
                        )

            for w in range(NWAVE):
                ca, cb = 2 * w, 2 * w + 1
                p1 = True  # ttr path crashes at runtime in this env
                for m in range(MT):
                    lhs = q_t[m // 4][:, (m % 4) * 128 : (m % 4 + 1) * 128]
                    qt = psum_q.tile([128, 1024], F32, tag="qq")
                    pt = psum_p.tile([128, 1024], F32, tag="pp")
                    nc.tensor.matmul(qt[:, 0:512], lhs, dd_ap(ca), start=True, stop=True)
                    nc.tensor.matmul(qt[:, 512:1024], lhs, dd_ap(cb), start=True, stop=True)
                    st = not p1
                    nc.tensor.matmul(pt[:, 0:512], lhs, ds_ap(ca), start=True, stop=st)
                    nc.tensor.matmul(pt[:, 512:1024], lhs, ds_ap(cb), start=True, stop=st)
                    aq = absq_p.tile([128, 1024], F32R, tag="aq")
                    if p1:
                        nc.scalar.activation(aq[:], qt[:], ABS)
                    else:
                        nc.scalar.copy(aq[:], qt[:])
                    if pend:
                        flush_pend()
                    pend.append((1 if p1 else 2, pt, aq, (w * MT + m) * 2))

            # paired neg term: fp32r fast-weight-load excludes PE column
            # tiling, so instead of partition-packing, broadcast a 4-row
            # block of q tokens as lhsT (full [128,512] matmuls; 3/4 of the
            # PE work is garbage cross-row products the epilogue ignores,
            # but PE has slack and abs/merge/reduce stay full width).
            # alloc a covers rows 2a, 2a+1: P(2a)|P(2a+1)|Q(2a)|Q(2a+1)
            for a in range(4):
                qt = psum_q.tile([128, 1024], F32, tag="qq")
                pt = psum_p.tile([128, 1024], F32, tag="pp")
                lhs = ql_t[:, (a // 2) * 128 : (a // 2 + 1) * 128]
                for g in range(2):
                    b = 2 * a + g
                    nc.tensor.matmul(
                        qt[:, g * H : (g + 1) * H],
                        lhs,
                        ndr[:, b * H : (b + 1) * H],
                        start=True,
                        stop=True,
                    )
                for g in range(2):
                    b = 2 * a + g
                    nc.tensor.matmul(
                        pt[:, g * H : (g + 1) * H],
                        lhs,
                        nsr[:, b * H : (b + 1) * H],
                        start=True,
                        stop=False,
                    )
                aq = absq_p.tile([128, 1024], F32R, tag="aq")
                nc.scalar.activation(aq[:], qt[:], ABS)
                if pend:
                    flush_pend()
                pend.append((1, pt, aq, MT * LB + 2 * a))
            while pend:
                flush_pend()

            nc.sync.dma_start(out[:], mx[:])

    nc.compile()
    return nc


def get_nc():
    if "nc" not in _NC_CACHE:
        _NC_CACHE["nc"] = _build_nc()
    return _NC_CACHE["nc"]


def _prep_inputs(q, d, nd):
    """Build the 8 per-core input maps."""
    qtok = np.ascontiguousarray(q.reshape(B * N, D).T)  # (128, 2048)
    iden = np.eye(128, dtype=np.float32)

    def halves(x):  # x: (B, S, D) -> (B, 512, D) sum/diff halves
        a = x[:, :H, :]
        b = x[:, H:, :]
        return (a + b) * np.float32(0.5), (a - b) * np.float32(0.5)

    hs, hd = halves(d)
    gs, gd = halves(nd)
    # waves 2-3 (local docs 4-7) use raw halves (copy+ttr path on device)
    ra, rb = d[:, :H, :], d[:, H:, :]

    def chunkT(x, r):  # (B,512,D) slice rows -> (128, 8*512)
        c = x[r * LB : (r + 1) * LB]  # (8, 512, 128)
        return np.ascontiguousarray(np.transpose(c, (2, 0, 1)).reshape(D, LB * H))

    def chunkT_mixed(sumf, rawf, r):  # sum/dif for local 0-3, raw for 4-7
        c = np.concatenate(
            [sumf[r * LB : r * LB + 4], rawf[r * LB + 4 : (r + 1) * LB]], axis=0
        )  # (8, 512, 128)
        return np.ascontiguousarray(np.transpose(c, (2, 0, 1)).reshape(D, LB * H))

    maps = []
    for r in range(NCORES):
        maps.append(
            {
                "qT": qtok,
                "qLocT": np.ascontiguousarray(
                    qtok[:, r * LB * N : (r + 1) * LB * N]
                ),
                "dsumT": chunkT(hs, r),
                "ddifT": chunkT(hd, r),
                "nsumT": chunkT(gs, r),
                "ndifT": chunkT(gd, r),
                "iden": iden,
            }
        )
    return maps


def _epilogue(blocks, offset):
    """blocks: list of 8 (128, OUT_COLS) arrays -> final loss (float32)."""
    S_mat = np.empty((B, B), dtype=np.float64)
    negs = np.empty(B, dtype=np.float64)
    for r in range(NCORES):
        blk = np.asarray(blocks[r], dtype=np.float64)
        # in-batch: col (w*MT + m)*2 + j -> doc c = 2w+j; partition p of
        # m-tile m -> token g = m*128+p -> (b = g//32, n = g%32)
        sc = blk[:, : MT * LB].reshape(128, NWAVE, MT, 2)  # p, w, m, j
        # token-sum: scores_tok[g, c] with g = m*128+p
        tok = np.transpose(sc, (2, 0, 1, 3)).reshape(B * N, LB)  # g, c
        S_mat[:, r * LB : (r + 1) * LB] = tok.reshape(B, N, LB).sum(axis=1)
        # neg: col MT*LB + 2a + g holds row b = 2a + g, valid only at
        # partitions [(b % 4) * 32 : +32] (rest is cross-row garbage)
        nsum = np.empty(LB)
        for b in range(LB):
            p0 = (b % 4) * N
            nsum[b] = blk[p0 : p0 + N, MT * LB + b].sum()
        negs[r * LB : (r + 1) * LB] = nsum

    pos = np.diag(S_mat)
    x = (negs - pos) / TEMP
    loss1 = np.logaddexp(0.0, x).mean()  # stable softplus

    logits = S_mat / TEMP
    # jnp.take_along_axis index semantics: negative indices wrap once,
    # out-of-range indices yield NaN (fill mode)
    raw = np.arange(B) + int(offset)
    idx = np.where(raw < 0, raw + B, raw)
    valid = (idx >= 0) & (idx < B)
    row_max = logits.max(axis=1, keepdims=True)
    lse = np.log(np.exp(logits - row_max).sum(axis=1, keepdims=True)) + row_max
    logp = logits - lse
    picked = logp[np.arange(B), np.clip(idx, 0, B - 1)]
    picked = np.where(valid, picked, np.nan)
    ce = -picked.mean()

    return np.float32((loss1 + ce) / 2.0)


def kernel(query_embeddings, doc_embeddings, neg_doc_embeddings, offset):
    from concourse.bass_utils import run_bass_kernel_spmd

    q = np.asarray(query_embeddings, dtype=np.float32)
    d = np.asarray(doc_embeddings, dtype=np.float32)
    nd = np.asarray(neg_doc_embeddings, dtype=np.float32)
    assert q.shape == (B, N, D) and d.shape == (B, S, D) and nd.shape == (B, S, D)

    nc = get_nc()
    maps = _prep_inputs(q, d, nd)
    res = run_bass_kernel_spmd(nc, maps, core_ids=list(range(NCORES)))
    blocks = [res.results[r]["out"] for r in range(NCORES)]
    return _epilogue(blocks, offset)


def run_traced(query_embeddings, doc_embeddings, neg_doc_embeddings, offset, **trace_kw):
    """Like kernel() but returns (loss, BassKernelResults) for profiling."""
    from concourse.bass_utils import run_bass_kernel_spmd

    q = np.asarray(query_embeddings, dtype=np.float32)
    d = np.asarray(doc_embeddings, dtype=np.float32)
    nd = np.asarray(neg_doc_embeddings, dtype=np.float32)
    nc = get_nc()
    maps = _prep_inputs(q, d, nd)
    res = run_bass_kernel_spmd(
        nc, maps, core_ids=list(range(NCORES)), trace=True, **trace_kw
    )
    blocks = [res.results[r]["out"] for r in range(NCORES)]
    return _epilogue(blocks, offset), res
